# revision 1
# baseline (speedup 1.0000x reference)
# Bidirectional multi-head attention (key-padding mask) on 8 Trainium2 cores.
#
# Sharding: core = (batch b, head-group hg); B=4 x 2 head-groups of 8 heads.
# Each core computes y_partial^T [C, T] for its 8 heads of batch b; host sums
# the two head-group partials per batch and transposes back to [T, C].
#
# Masked keys are gathered away on the host (exactly equivalent to the -inf
# mask: masked keys contribute 0 attention weight), so the device only
# attends over ~half the keys, padded to a multiple of 128 with bias -30000
# (exp underflows to exactly 0).
#
# Device layouts (partition dim first):
#   QT_all  [128, 4, T]   q^T per head-pair tile (heads 2j,2j+1 on 64+64 parts)
#   KT_all  [128, 4, TK]  k^T likewise
#   V_sb    [128, KT_tiles, 8, 65] bf16: V rows (key on partitions) + ones col
#   S^T     = matmul(lhsT=KT slice [64,128], rhs=QT [64,512]) -> [128 keys, q]
#   exp     fused on ScalarE: exp(s/8 + bias_k), bias per key partition
#   attn@V  = matmul(lhsT=V_aug [128,65], rhs=attnT [128,512]) accum over kt
#             row 64 of the psum = per-query row-sum (ones column trick)
#   norm    reciprocal(rowsum) broadcast across 64 partitions via a K=1
#             ones outer-product matmul, then DVE multiply
#   proj    y^T = matmul(lhsT=WpT tile, rhs=outT) accum over 4 c' tiles

import sys

import ml_dtypes
import numpy as np

try:
    import concourse.bacc as bacc  # noqa: F401
except ImportError:
    sys.path.insert(0, "/opt/trn_rl_repo")

import concourse.bacc as bacc
import concourse.bass as bass
import concourse.mybir as mybir
import concourse.tile as tile
from concourse.bass_interp import get_hw_module
from concourse.bass_utils import run_bass_kernel_spmd

F32 = mybir.dt.float32
F32R = mybir.dt.float32r
BF16 = mybir.dt.bfloat16
P = 128

D_MODEL = 1024
N_HEADS = 16
HEAD_DIM = 64
B = 4
T_FULL = 2048
HL = 8  # heads per core
PAIRS = HL // 2
CT = D_MODEL // P  # c tiles


def _chunks(total, size):
    out = []
    s = 0
    while s < total:
        out.append((s, min(size, total - s)))
        s += size
    return out


def build_program(T=T_FULL, TK=1152, rounds=1):
    """Build the per-core Bass program. Same program runs on all 8 cores."""
    assert T % 1024 == 0 and TK % P == 0
    KTT = TK // P  # key tiles
    D = HEAD_DIM
    HT = T // 2  # half of queries; exp batch + attnT tile granularity

    nc = bacc.Bacc("TRN2", target_bir_lowering=False, debug=False, num_devices=1)

    xT = nc.dram_tensor("xT", [D_MODEL, T], BF16, kind="ExternalInput")
    xkT = nc.dram_tensor("xkT", [D_MODEL, TK], BF16, kind="ExternalInput")
    WqT = nc.dram_tensor("WqT", [D_MODEL, HL * D], BF16, kind="ExternalInput")
    WkT = nc.dram_tensor("WkT", [D_MODEL, HL * D], BF16, kind="ExternalInput")
    WvT = nc.dram_tensor("WvT", [D_MODEL, HL * D], BF16, kind="ExternalInput")
    WpT = nc.dram_tensor("WpT", [HL * D, D_MODEL], F32, kind="ExternalInput")
    kbias = nc.dram_tensor("kbias", [P, KTT], F32, kind="ExternalInput")
    yT = nc.dram_tensor("yT", [D_MODEL, T], F32, kind="ExternalOutput")

    xT_r = xT.ap().rearrange("(ct p) t -> p ct t", p=P)
    xkT_r = xkT.ap().rearrange("(ct p) t -> p ct t", p=P)
    WqT_r = WqT.ap().rearrange("(ct p) o -> p ct o", p=P)
    WkT_r = WkT.ap().rearrange("(ct p) o -> p ct o", p=P)
    WvT_r = WvT.ap().rearrange("(ct p) o -> p ct o", p=P)
    WpT_r = WpT.ap().rearrange("(ct p) o -> p ct o", p=P)
    yT_r = yT.ap().rearrange("(mt p) t -> p mt t", p=P)

    with tile.TileContext(nc) as tc:
        for _round in range(rounds):
            with tc.tile_pool(name="persist", bufs=1) as pers:
                KT_sb = pers.tile([P, PAIRS, TK], BF16, tag="KTall")
                V_sb = pers.tile([P, KTT, HL, D + 1], BF16, tag="Vall")
                QT_sb = pers.tile([P, PAIRS, T], BF16, tag="QTall")
                outT_sb = pers.tile([P, PAIRS, T], F32R, tag="outT")
                kbias_sb = pers.tile([P, KTT], F32, tag="kbias")
                ones_f = pers.tile([1, D], F32, tag="onesf")
                ones_sb = pers.tile([1, D], F32R, tag="ones")

                nc.sync.dma_start(kbias_sb[:], kbias.ap())
                nc.gpsimd.memset(ones_f[:], 1.0)
                nc.vector.tensor_copy(out=ones_sb[:], in_=ones_f[:])
                nc.gpsimd.memset(V_sb[:, :, :, D : D + 1], 1.0)

                # ---- Phases KV + Q: projections, x streamed in 512-col chunks ----
                with (
                    tc.tile_pool(name="xstream", bufs=3) as xs,
                    tc.tile_pool(name="qw", bufs=1) as qw,
                    tc.tile_pool(name="psA", bufs=4, space="PSUM") as psA,
                ):
                    WqT_sb = qw.tile([P, CT, HL * D], BF16, tag="WqT")
                    with tc.tile_pool(name="kvw", bufs=1) as kvw:
                        WkT_sb = kvw.tile([P, CT, HL * D], BF16, tag="WkT")
                        WvT_sb = kvw.tile([P, CT, HL * D], BF16, tag="WvT")
                        first_xc = xs.tile([P, CT, 512], BF16, tag="xc", name="xc")
                        w0 = min(512, TK)
                        # interleave strips: K matmul on c-tile ct needs only
                        # (WkT strip ct, xc strip ct) — compute starts early
                        for ct in range(CT):
                            nc.sync.dma_start(WkT_sb[:, ct, :], WkT_r[:, ct, :])
                            nc.sync.dma_start(first_xc[:, ct, :w0], xkT_r[:, ct, 0:w0])
                        for ct in range(CT):
                            nc.sync.dma_start(WvT_sb[:, ct, :], WvT_r[:, ct, :])
                        for ct in range(CT):
                            nc.sync.dma_start(WqT_sb[:, ct, :], WqT_r[:, ct, :])

                        for s, w in _chunks(TK, 512):
                            if s == 0:
                                xc = first_xc
                            else:
                                xc = xs.tile([P, CT, 512], BF16, tag="xc", name="xc")
                                for ct in range(CT):
                                    nc.sync.dma_start(
                                        xc[:, ct, :w], xkT_r[:, ct, s : s + w]
                                    )
                            # K^T for this key chunk, all 4 pair tiles
                            for m in range(PAIRS):
                                ps = psA.tile([P, 512], F32, tag="ps")
                                for ct in range(CT):
                                    nc.tensor.matmul(
                                        ps[:, :w],
                                        lhsT=WkT_sb[:, ct, m * P : (m + 1) * P],
                                        rhs=xc[:, ct, :w],
                                        start=(ct == 0),
                                        stop=(ct == CT - 1),
                                    )
                                nc.vector.tensor_copy(
                                    out=KT_sb[:, m, s : s + w], in_=ps[:, :w]
                                )
                            # V rows for this chunk's key tiles
                            for tl in range(w // P):
                                tt = s // P + tl
                                ps = psA.tile([P, 512], F32, tag="ps")
                                for ct in range(CT):
                                    nc.tensor.matmul(
                                        ps[:],
                                        lhsT=xc[:, ct, tl * P : (tl + 1) * P],
                                        rhs=WvT_sb[:, ct, :],
                                        start=(ct == 0),
                                        stop=(ct == CT - 1),
                                    )
                                nc.vector.tensor_copy(
                                    out=V_sb[:, tt, :, 0:D],
                                    in_=ps[:].rearrange("p (h d) -> p h d", h=HL),
                                )

                    for s, w in _chunks(T, 512):
                        xc = xs.tile([P, CT, 512], BF16, tag="xc", name="xc")
                        for ct in range(CT):
                            nc.sync.dma_start(xc[:, ct, :w], xT_r[:, ct, s : s + w])
                        for m in range(PAIRS):
                            ps = psA.tile([P, 512], F32, tag="ps")
                            for ct in range(CT):
                                nc.tensor.matmul(
                                    ps[:, :w],
                                    lhsT=WqT_sb[:, ct, m * P : (m + 1) * P],
                                    rhs=xc[:, ct, :w],
                                    start=(ct == 0),
                                    stop=(ct == CT - 1),
                                )
                            nc.vector.tensor_copy(
                                out=QT_sb[:, m, s : s + w], in_=ps[:, :w]
                            )

                # ---- Phases attn + proj (shared pools so proj overlaps the tail) ----
                with (
                    tc.tile_pool(name="attn", bufs=1) as ap_,
                    tc.tile_pool(name="nrm", bufs=2) as nrm,
                    tc.tile_pool(name="drp", bufs=2, space="DRAM") as dp,
                    tc.tile_pool(name="proj", bufs=1) as pp,
                    tc.tile_pool(name="ysb", bufs=6) as yp,
                    tc.tile_pool(name="ps_st", bufs=1, space="PSUM") as ps_st,
                    tc.tile_pool(name="ps_av", bufs=4, space="PSUM") as ps_av,
                ):
                    # WpT strips load during attn; fresh addresses, no WAR on attn pools
                    wp = []
                    for ct in range(PAIRS):
                        w_t = pp.tile([P, D_MODEL], F32R, tag=f"wp{ct}", name=f"wp{ct}")
                        nc.sync.dma_start(w_t[:], WpT_r[:, ct, :].bitcast(F32R))
                        wp.append(w_t)

                    def emit_st(pair, att):
                        # scores^T + exp. Lane order (hf, hh, kt) with psum
                        # slot by kt-parity: ACT streams a whole lane while
                        # the other slot's matmul refills, and lane order
                        # matches the order AV(pair-1) releases attn tiles.
                        for hf, s in ((0, 0), (1, HT)):
                            for hh in (0, 1):
                                base = hh * 64
                                for kt in range(KTT):
                                    pst = ps_st.tile([P, HT], F32, tag=f"st{kt % 2}")
                                    for s2, w2 in _chunks(HT, 512):
                                        nc.tensor.matmul(
                                            pst[:, s2 : s2 + w2],
                                            lhsT=KT_sb[
                                                base : base + 64,
                                                pair,
                                                kt * P : (kt + 1) * P,
                                            ],
                                            rhs=QT_sb[
                                                base : base + 64,
                                                pair,
                                                s + s2 : s + s2 + w2,
                                            ],
                                            start=True,
                                            stop=True,
                                        )
                                    nc.scalar.activation(
                                        att[hh][hf][:, kt, :],
                                        pst[:],
                                        mybir.ActivationFunctionType.Exp,
                                        bias=kbias_sb[:, kt : kt + 1],
                                        scale=0.125,
                                    )

                    def emit_av(pair, att):
                        # attn @ V_aug + normalize
                        for hh in (0, 1):
                            h = 2 * pair + hh
                            for s, w in _chunks(T, 512):
                                hf, so = (0, s) if s < HT else (1, s - HT)
                                av = ps_av.tile([P, 512], F32, tag="av")
                                for kt in range(KTT):
                                    nc.tensor.matmul(
                                        av[0 : D + 1, :w],
                                        lhsT=V_sb[:, kt, h, :],
                                        rhs=att[hh][hf][:, kt, so : so + w],
                                        start=(kt == 0),
                                        stop=(kt == KTT - 1),
                                    )
                                rc = nrm.tile([1, 512], F32, tag="rc")
                                nc.vector.reciprocal(rc[:, :w], av[D : D + 1, :w])
                                rcd = dp.tile([1, 512], F32, tag="rcd")
                                nc.sync.dma_start(rcd[:, :w], rc[:, :w])
                                rcb = nrm.tile([64, 512], F32, tag="rcb")
                                nc.sync.dma_start(
                                    rcb[:, :w], rcd[0:1, :w].to_broadcast((64, w))
                                )
                                nc.vector.tensor_mul(
                                    out=outT_sb[
                                        hh * 64 : (hh + 1) * 64, pair, s : s + w
                                    ],
                                    in0=av[0:D, :w],
                                    in1=rcb[:, :w],
                                )


                    atts = []
                    for pair in range(PAIRS):
                        atts.append([
                            [
                                ap_.tile(
                                    [P, KTT, HT], BF16, tag=f"attn{hh}{hf}",
                                    name=f"attn{hh}{hf}",
                                )
                                for hf in (0, 1)
                            ]
                            for hh in (0, 1)
                        ])
                        emit_st(pair, atts[pair])
                        if pair > 0:
                            emit_av(pair - 1, atts[pair - 1])
                    emit_av(PAIRS - 1, atts[PAIRS - 1])

                    # proj: psums share the st slots (free once last exps retire)
                    pcyc = 0
                    for s, w in _chunks(T, 512):
                        for m in range(D_MODEL // P):
                            ptag = ("st0", "st1", "av")[pcyc % 3]
                            pool_ = ps_av if ptag == "av" else ps_st
                            ps = pool_.tile([P, 512], F32, tag=ptag)
                            pcyc += 1
                            for ct in range(PAIRS):
                                nc.tensor.matmul(
                                    ps[:, :w],
                                    lhsT=wp[ct][:, m * P : (m + 1) * P],
                                    rhs=outT_sb[:, ct, s : s + w],
                                    start=(ct == 0),
                                    stop=(ct == PAIRS - 1),
                                )
                            ysb = yp.tile([P, 512], F32, tag="ysb")
                            nc.vector.tensor_copy(out=ysb[:, :w], in_=ps[:, :w])
                            nc.sync.dma_start(yT_r[:, m, s : s + w], ysb[:, :w])

    return nc


def prep_core_inputs(x, pad_mask, W_qkv, W_proj, b, hg, TK):
    """Host-side shard prep for core (b, hg)."""
    C = D_MODEL
    D = HEAD_DIM
    xb = np.asarray(x[b], dtype=np.float32)  # [T, C]
    mask = np.asarray(pad_mask[b])
    idx = np.nonzero(~mask)[0]
    cnt = len(idx)
    assert cnt <= TK, f"key count {cnt} exceeds TK={TK}"

    BF = ml_dtypes.bfloat16
    xT = np.ascontiguousarray(xb.T).astype(BF)  # [C, T]
    xkT = np.zeros((C, TK), dtype=BF)
    xkT[:, :cnt] = xb[idx].T.astype(BF)

    kb = np.zeros((TK,), dtype=np.float32)
    kb[cnt:] = -30000.0
    kbias = np.ascontiguousarray(kb.reshape(TK // P, P).T)  # [128, KTT]

    Wq = W_qkv[0:C].reshape(N_HEADS, D, C)
    Wk = W_qkv[C : 2 * C].reshape(N_HEADS, D, C)
    Wv = W_qkv[2 * C : 3 * C].reshape(N_HEADS, D, C)
    heads = range(hg * HL, (hg + 1) * HL)
    WqT = np.ascontiguousarray(np.concatenate([Wq[h] for h in heads], axis=0).T)
    WkT = np.ascontiguousarray(np.concatenate([Wk[h] for h in heads], axis=0).T)
    WvT = np.ascontiguousarray(np.concatenate([Wv[h] for h in heads], axis=0).T)
    WpT = np.ascontiguousarray(
        np.concatenate([W_proj[:, h * D : (h + 1) * D] for h in heads], axis=1).T
    )
    return {
        "xT": xT,
        "xkT": xkT,
        "WqT": WqT.astype(BF),
        "WkT": WkT.astype(BF),
        "WvT": WvT.astype(BF),
        "WpT": WpT.astype(np.float32),
        "kbias": kbias,
    }


def kernel(x, pad_mask, W_qkv, W_proj):
    x = np.asarray(x, dtype=np.float32)
    pad_mask = np.asarray(pad_mask, dtype=bool)
    W_qkv = np.asarray(W_qkv, dtype=np.float32)
    W_proj = np.asarray(W_proj, dtype=np.float32)
    Bv, T, C = x.shape

    counts = (~pad_mask).sum(axis=1)
    TK = max(int(-(-counts.max() // P)) * P, P)

    nc = build_program(T=T, TK=TK)
    nc.compile()
    nc.m = get_hw_module(nc.m)

    in_maps = []
    for c in range(8):
        b, hg = c // 2, c % 2
        in_maps.append(prep_core_inputs(x, pad_mask, W_qkv, W_proj, b, hg, TK))

    res = None
    for attempt in range(3):
        try:
            res = run_bass_kernel_spmd(nc, in_maps, core_ids=list(range(8)))
            break
        except Exception:
            if attempt == 2:
                raise
            import time as _time

            _time.sleep(5.0)

    y = np.empty((Bv, T, C), dtype=np.float32)
    for b in range(Bv):
        yT = res.results[2 * b]["yT"] + res.results[2 * b + 1]["yT"]
        y[b] = yT.T
    return y

